# revision 37
# baseline (speedup 1.0000x reference)
"""Multi-head attention (dense_transformer) Trainium2 Bass kernel.

Problem: x[8, 512, 32, 32]; per-batch 1x1-conv QKV projections, 8-head
attention over N=H*W=1024 positions (head_dim 64), output projection,
residual. Sharding: data-parallel over batch B=8 across the 8 cores -
one batch element per core, no collectives.

v2: fp8e4 + DoubleRow matmuls everywhere (2x128 contraction per
instruction at 0.5 PE cycles/row -> 3x less PE time than bf16), softmax
exp split across the Activation engine (native Exp) and GPSIMD
(tensor_tensor pow with base e^(0.125/256), so the logit scale rides in
the base), and all bias work folded away:
  - bk dropped exactly (softmax is invariant to per-query logit shifts),
  - bq folded into the Q PSUM->SBUF cast (per-partition tensor_scalar),
  - bv folded via attention(V + bv) = attention(V) + bv into the
    residual: x32 = x + bo + Wo @ bv is prepared on the host,
  - weights are host-scaled by 16 to sit in fp8e4's sweet spot; the
    VT ones column (2^-6) and the final output scale 2^-14 undo it.

Per-core dataflow (all matmul operands fp8e4, fp32 PSUM accumulate):
  - Q8/K8 in DoubleRow layout [128, 2, N]: partition 32q+r, group g of
    head-group tile t holds channel o = 64*(4t + r//32) + 32g + (r%32)
    (host permutes Wq/Wk columns so the projection writes this layout
    directly); the same layout serves the S^T matmul with the d=64
    contraction as [32 partitions x 2 groups] at PE tile base 32q.
  - S^T per (head, jt): one DR matmul per 512 columns; exp (Act) or
    pow (Pool) -> P8 [128, jt, N] fp8.
  - AV: VT [128, jt, h, 64+ones] fp8; 4 DR matmuls accumulate [65, 512];
    row 64 = 2^-6 * colsum(P). Denominators bounce through DRAM per
    head ([2,512] -> [128,8] reciprocal -> broadcast [64,512]), then one
    tensor_tensor mult normalizes straight out of PSUM into O16 fp8.
  - Output projection: DR over O16 [128, 4, N]; epilogue is a single
    scalar_tensor_tensor (psum * 2^-14) + x32 per [128,512] tile.
"""

import sys

if "/opt/trn_rl_repo" not in sys.path:
    sys.path.insert(0, "/opt/trn_rl_repo")

import numpy as np
import ml_dtypes

import concourse.bass as bass
import concourse.mybir as mybir
from concourse.tile import TileContext

DIM = 512
NH = 8
HD = 64
N = 1024
P = 128
CT = DIM // P  # 4 c-tiles of 128 channels
JT = N // P    # 8 j-tiles of 128 positions
F32 = mybir.dt.float32
BF16 = mybir.dt.bfloat16
FP8 = mybir.dt.float8e4
AOP = mybir.AluOpType
EXP = mybir.ActivationFunctionType.Exp
DR = mybir.MatmulPerfMode.DoubleRow

WS = 16.0                      # host weight scale (fp8 range)
SEXP = 0.125 / (WS * WS)       # exp scale: 1/sqrt(64) / (16*16)
ONES_VAL = 2.0 ** -6           # denominator lhsT -> O16 = 1024*attn(V)
OUT_SCALE = 2.0 ** -14         # undo 16(Wo) * 1024(O16) / 16(V)... = 2^14

# Schraudolph exp on DVE: trunc(S*SCHRA_A + SCHRA_B) as int8 IS
# exp(S*SEXP) in fp8e4m3 bits (GPSIMD can't read PSUM; DVE has no exp -
# but one tensor_scalar mult+add into an int8 bitcast view is enough).
# B tuned for min mean |rel err| (2.6%) under truncation semantics.
SCHRA_A = float(8 * np.log2(np.e) * SEXP)
SCHRA_B = 55.58  # HW rounds the f32->int8 convert; CoreSim truncates

# exp engine split: 39 of 64 units on Act, 25 on DVE-Schraudolph
# (Bresenham spread). Act additionally absorbs the Q/K/V PSUM->SBUF
# casts (exp/copy/identity share one activation table - no reloads),
# which balances Act ~53us / DVE ~53us.
ACT_N = 43
ACT_PAT = tuple(
    (i * ACT_N) // 64 != ((i + 1) * ACT_N) // 64 for i in range(64)
)


class FixedTileContext(TileContext):
    """Works around a walrus/bass snapshot mismatch: this walrus build
    accepts only one sync-wait command per instruction, but Tile's wait
    assigner happily attaches several. After scheduling, excess waits on
    any instruction are peeled off onto same-engine NOPs inserted right
    before it (same blocking semantics: the engine executes in order)."""

    MAX_WAITS = 1
    MAX_WAITS_DATA = 1
    _wsplit_ctr = 0

    def _split_sync_waits(self):
        seq_only = mybir.SEQUENCER_ONLY_OPCODES
        for fn in self.nc.m.functions:
            for blk in fn.blocks:
                insts = list(blk.instructions)
                out = []
                for inst in insts:
                    si = inst.sync_info
                    limit = (
                        self.MAX_WAITS
                        if inst.opcode in seq_only
                        else self.MAX_WAITS_DATA
                    )
                    if si is not None and len(si.on_wait) > limit:
                        waits = list(si.on_wait)
                        movers = waits[:-limit]
                        keep = waits[-limit:]
                        del si.on_wait[:]
                        for w in keep:
                            si.on_wait.append(w)
                        for w in movers:
                            FixedTileContext._wsplit_ctr += 1
                            nop = mybir.InstNoOp(
                                name=f"wsplit-{FixedTileContext._wsplit_ctr}",
                                ins=[],
                                outs=[],
                            )
                            nop.engine = inst.engine
                            nop.sync_info = mybir.SyncInfo(on_wait=[w], on_update=[])
                            out.append(nop)
                    out.append(inst)
                if len(out) != len(insts):
                    del blk.instructions[:]
                    for i in out:
                        blk.add_instruction(i)

    split_on_exit = True

    def __exit__(self, *exc):
        ret = super().__exit__(*exc)
        if exc[0] is None and self.split_on_exit:
            self._split_sync_waits()
        return ret


def build_nc(split_waits=True):
    nc = bass.Bass()

    x8d = nc.dram_tensor("x8", [P, CT, N], FP8, kind="ExternalInput")
    x32d = nc.dram_tensor("x32", [DIM, N], F32, kind="ExternalInput")
    wqd = nc.dram_tensor("wq", [P, CT, DIM], FP8, kind="ExternalInput")
    wkd = nc.dram_tensor("wk", [P, CT, DIM], FP8, kind="ExternalInput")
    wvd = nc.dram_tensor("wv", [P, CT, DIM], FP8, kind="ExternalInput")
    wod = nc.dram_tensor("wo", [P, CT, DIM], FP8, kind="ExternalInput")
    bqd = nc.dram_tensor("bq", [P, CT], F32, kind="ExternalInput")
    outd = nc.dram_tensor("out", [DIM, N], F32, kind="ExternalOutput")

    FixedTileContext.split_on_exit = split_waits
    with FixedTileContext(nc) as tc:
        with (
            tc.tile_pool(name="persist", bufs=1) as persist,
            tc.tile_pool(name="p8pool", bufs=3) as p8pool,
            tc.tile_pool(name="small", bufs=4) as small,
            tc.tile_pool(name="rbpool", bufs=3) as rbpool,
            tc.tile_pool(name="otile", bufs=4) as otile,
            tc.tile_pool(name="dram", bufs=1, space="DRAM") as dram,
            tc.tile_pool(name="psS", bufs=3, space="PSUM") as psS_pool,
        ):
            # ---------- persistent SBUF tensors ----------
            x8_sb = persist.tile([P, CT, N], FP8, tag="x8", name="x8")
            wq_sb = persist.tile([P, CT, DIM], FP8, tag="wq", name="wq")
            wk_sb = persist.tile([P, CT, DIM], FP8, tag="wk", name="wk")
            wv_sb = persist.tile([P, CT, DIM], FP8, tag="wv", name="wv")
            wo_sb = persist.tile([P, CT, DIM], FP8, tag="wo", name="wo")
            bq_sb = persist.tile([P, CT], F32, tag="bq", name="bq")
            Q8 = [
                persist.tile([P, 2, N], FP8, tag=f"q8_{t}", name=f"q8_{t}")
                for t in range(2)
            ]
            K8 = [
                persist.tile([P, 2, N], FP8, tag=f"k8_{t}", name=f"k8_{t}")
                for t in range(2)
            ]
            VT = persist.tile([P, JT, NH, HD], FP8, tag="vt", name="vt")
            ones64 = persist.tile([P, 2, HD], FP8, tag="ones64", name="ones64")
            O16 = persist.tile([P, CT, N], FP8, tag="o16", name="o16")
            xs32 = [
                persist.tile([P, N], F32, tag=f"x32_{t}", name=f"x32_{t}")
                for t in range(CT)
            ]

            # ---------- input loads ----------
            # sync queue: x8 (needed first; split so the first projection
            # matmuls only wait for the first half)
            nc.sync.dma_start(out=x8_sb[:, 0:2, :], in_=x8d[:, 0:2, :])
            nc.sync.dma_start(out=x8_sb[:, 2:4, :], in_=x8d[:, 2:4, :])
            # scalar queue (idle until first exp): V first (V-projection
            # leads so its copies warm the Act engine), then Q/K
            nc.scalar.dma_start(out=wk_sb, in_=wkd[:])
            nc.scalar.dma_start(out=wv_sb, in_=wvd[:])
            nc.scalar.dma_start(out=wq_sb, in_=wqd[:])
            nc.scalar.dma_start(out=bq_sb, in_=bqd[:])
            # gpsimd queue (cheap issue): the rest
            nc.gpsimd.dma_start(out=wo_sb, in_=wod[:])
            x32r = x32d.rearrange("(t p) n -> t p n", p=P)
            for t in range(CT):
                nc.gpsimd.dma_start(out=xs32[t], in_=x32r[t])

            # warm the exp table on Act; fill the pow base tile on Pool
            warm = small.tile([1, 8], F32, tag="warm", name="warm")
            nc.vector.memset(warm, 0.0)
            nc.scalar.activation(warm, warm, EXP)
            # denominator matmul lhsT (value 2^-6, exact in fp8): broadcasts
            # the P colsum across 64 output partitions
            nc.vector.memset(ones64, ONES_VAL)

            # ---------- exp unit emission (Act / Pool split) ----------
            exp_ctr = [0]

            def exp_unit(ps, p8t, jt):
                u = exp_ctr[0]
                exp_ctr[0] += 1
                if ACT_PAT[u % len(ACT_PAT)]:
                    nc.scalar.activation(p8t[:, jt, :], ps, EXP, scale=SEXP)
                else:
                    nc.vector.tensor_scalar(
                        p8t[:, jt, :].bitcast(mybir.dt.int8),
                        ps,
                        SCHRA_A,
                        SCHRA_B,
                        AOP.mult,
                        AOP.add,
                    )

            def s_head(h, p8t):
                """S^T + exp for head h -> P8 tile [P, JT, N]."""
                t, q = divmod(h, 4)
                b0 = 32 * q
                for jt in range(JT):
                    ps = psS_pool.tile([P, N], F32, tag="psS", name="psS")
                    for ih in range(2):
                        nc.tensor.matmul(
                            ps[:, ih * 512 : (ih + 1) * 512],
                            lhsT=K8[t][b0 : b0 + 32, :, jt * P : (jt + 1) * P],
                            rhs=Q8[t][b0 : b0 + 32, :, ih * 512 : (ih + 1) * 512],
                            start=True,
                            stop=True,
                            perf_mode=DR,
                            tile_position=(b0, 0),
                        )
                    exp_unit(ps, p8t, jt)

            def av_mm(dst, lhsT, p8t, ih, jp):
                nc.tensor.matmul(
                    dst[:, ih * 512 : (ih + 1) * 512],
                    lhsT=lhsT,
                    rhs=p8t[:, 2 * jp : 2 * jp + 2, ih * 512 : (ih + 1) * 512],
                    start=(jp == 0),
                    stop=(jp == JT // 2 - 1),
                    perf_mode=DR,
                    skip_group_check=True,
                )

            def head_slot(h_s, p8s, h_av, p8a, psO_pool):
                """One pipeline slot: S^T+exp for head h_s interleaved at
                jt granularity with the AV work of head h_av (2 AV matmuls
                ride between consecutive exp units so the PE never detours
                long enough to starve the exp engines).

                AV per head: DoubleRow matmuls may only target PSUM
                partition base 0, so each head gets per-i-half [64, 512] AV
                tiles plus [64, 512] denominator tiles where the ones64
                matmul replicates the P colsum across all 64 output
                partitions (same free-size cost). Denominator chunks go
                first (reciprocals run mid-slot, PSUM->SBUF, already in the
                shape the normalize mults want - no DRAM bounce anywhere).
                The 1-bank tiles keep psO at 2 banks so psS can triple
                buffer, hiding the PE+semaphore latency between exp units."""
                if h_av is not None:
                    pr, hh = divmod(h_av, 2)
                    pdt = [None, None]
                    pot = [None, None]
                    rbt = [
                        rbpool.tile([HD, 512], F32, tag="rb", name="rb")
                        for _ in range(2)
                    ]
                if h_s is not None:
                    t, q = divmod(h_s, 4)
                    b0 = 32 * q
                for jt in range(JT):
                    if h_s is not None:
                        ps = psS_pool.tile([P, N], F32, tag="psS", name="psS")
                        for ih in range(2):
                            nc.tensor.matmul(
                                ps[:, ih * 512 : (ih + 1) * 512],
                                lhsT=K8[t][b0 : b0 + 32, :,
                                           jt * P : (jt + 1) * P],
                                rhs=Q8[t][b0 : b0 + 32, :,
                                          ih * 512 : (ih + 1) * 512],
                                start=True,
                                stop=True,
                                perf_mode=DR,
                                tile_position=(b0, 0),
                            )
                        exp_unit(ps, p8s, jt)
                    if h_av is not None:
                        is_po = jt >= 4
                        ih, half = divmod(jt - 4 if is_po else jt, 2)
                        if jt == 2:
                            nc.vector.reciprocal(rbt[0], pdt[0])
                        elif jt == 4:
                            nc.vector.reciprocal(rbt[1], pdt[1])
                        elif jt == 6:
                            nc.vector.tensor_tensor(
                                O16[hh * HD : (hh + 1) * HD, pr, 0:512],
                                pot[0],
                                rbt[0],
                                AOP.mult,
                            )
                        if half == 0:
                            tgt = psO_pool.tile(
                                [HD, 512], F32, tag="psO",
                                name="po" if is_po else "pd",
                            )
                            (pot if is_po else pdt)[ih] = tgt
                        for jp in (2 * half, 2 * half + 1):
                            nc.tensor.matmul(
                                (pot if is_po else pdt)[ih],
                                lhsT=(VT[:, 2 * jp : 2 * jp + 2, h_av, :]
                                      if is_po else ones64),
                                rhs=p8a[:, 2 * jp : 2 * jp + 2,
                                        ih * 512 : (ih + 1) * 512],
                                start=(jp == 0),
                                stop=(jp == JT // 2 - 1),
                                perf_mode=DR,
                                skip_group_check=True,
                            )
                if h_av is not None:
                    nc.vector.tensor_tensor(
                        O16[hh * HD : (hh + 1) * HD, pr, 512:1024],
                        pot[1],
                        rbt[1],
                        AOP.mult,
                    )

            with tc.tile_pool(name="pp", bufs=2, space="PSUM") as pp:
                # ------ Q/K projections for head-group t: DR layout
                # ([128, 512] psum tiles keep this pool at 2 banks so psS
                # can triple-buffer during the whole kernel)
                def project_qk(w_sb, dst, t, bias):
                    for g in range(2):
                        for nh in range(2):
                            ps = pp.tile([P, 512], F32, tag="pp", name="pp")
                            for a in range(2):
                                nc.tensor.matmul(
                                    ps,
                                    lhsT=w_sb[
                                        :, 2 * a : 2 * a + 2,
                                        256 * t + 128 * g : 256 * t + 128 * g + 128,
                                    ],
                                    rhs=x8_sb[
                                        :, 2 * a : 2 * a + 2,
                                        nh * 512 : (nh + 1) * 512,
                                    ],
                                    start=(a == 0),
                                    stop=(a == 1),
                                    perf_mode=DR,
                                )
                            if bias is not None:
                                nc.scalar.add(
                                    dst[:, g, nh * 512 : (nh + 1) * 512],
                                    ps,
                                    bias[:, 2 * t + g : 2 * t + g + 1],
                                )
                            else:
                                nc.vector.tensor_copy(
                                    dst[:, g, nh * 512 : (nh + 1) * 512], ps
                                )

                project_qk(wk_sb, K8[0], 0, None)

                # ------ V projection -> VT [P, jt, h, d]
                for jt in range(JT):
                    ps = pp.tile([P, 512], F32, tag="pp", name="ppv")
                    for a in range(2):
                        nc.tensor.matmul(
                            ps,
                            lhsT=x8_sb[:, 2 * a : 2 * a + 2, jt * P : (jt + 1) * P],
                            rhs=wv_sb[:, 2 * a : 2 * a + 2, :],
                            start=(a == 0),
                            stop=(a == 1),
                            perf_mode=DR,
                        )
                    nc.scalar.copy(
                        VT[:, jt, :, :],
                        ps.rearrange("p (h d) -> p h d", h=NH),
                    )

                project_qk(wq_sb, Q8[0], 0, bq_sb)

                # heads 0, 1 S+exp early: gets Act/Pool going while the
                # remaining projections stream on the PE
                P8 = {}
                P8[0] = p8pool.tile([P, JT, N], FP8, tag="p8", name="p8")
                s_head(0, P8[0])
                P8[1] = p8pool.tile([P, JT, N], FP8, tag="p8", name="p8")
                s_head(1, P8[1])

                project_qk(wq_sb, Q8[1], 1, bq_sb)
                project_qk(wk_sb, K8[1], 1, None)

            # ---------- attention head pipeline ----------
            with tc.tile_pool(name="psO", bufs=2, space="PSUM") as psO_pool:
                for slot in range(8):
                    h_s = slot + 2 if slot < 6 else None
                    if h_s is not None:
                        P8[h_s] = p8pool.tile(
                            [P, JT, N], FP8, tag="p8", name="p8"
                        )
                    head_slot(
                        h_s,
                        P8.get(h_s),
                        slot,
                        P8[slot],
                        psO_pool,
                    )

            # ---------- output projection + residual ----------
            outr = outd.rearrange("(t p) n -> t p n", p=P)

            def out_proj(ot, ps):
                for nh in range(2):
                    for g in range(2):
                        nc.tensor.matmul(
                            ps[:, nh * 512 : (nh + 1) * 512],
                            lhsT=wo_sb[:, 2 * g : 2 * g + 2,
                                       ot * P : (ot + 1) * P],
                            rhs=O16[:, 2 * g : 2 * g + 2,
                                    nh * 512 : (nh + 1) * 512],
                            start=(g == 0),
                            stop=(g == 1),
                            perf_mode=DR,
                        )
                ob = otile.tile([P, N], F32, tag="ob", name="ob")
                nc.vector.scalar_tensor_tensor(
                    ob, ps, OUT_SCALE, xs32[ot], AOP.mult, AOP.add
                )
                dmae = (nc.scalar, nc.sync, nc.gpsimd, nc.sync)[ot]
                dmae.dma_start(out=outr[ot], in_=ob)

            with tc.tile_pool(name="po3", bufs=1, space="PSUM") as po3:
                for ot in range(3):
                    out_proj(ot, psS_pool.tile([P, N], F32, tag="psS",
                                               name=f"ps_o{ot}"))
                out_proj(3, po3.tile([P, N], F32, tag="op34", name="ps_o3"))
    return nc


_BF = ml_dtypes.bfloat16
_F8 = ml_dtypes.float8_e4m3


def _prep_maps(x, Wq, bq, Wk, bk, Wv, bv, Wo, bo):
    # plain numpy up front: inputs may arrive as jax device arrays and
    # transforming those would trigger on-device jax execution
    x, Wq, bq, Wk, bk, Wv, bv, Wo, bo = (
        np.asarray(a, np.float32) for a in (x, Wq, bq, Wk, bk, Wv, bv, Wo, bo)
    )
    B, C, H, W = x.shape
    xf = np.ascontiguousarray(x.reshape(B, C, H * W))
    rconst = bo + Wo @ bv  # residual constant: bo + Wo @ bv

    r_ = np.arange(P)
    cols = np.concatenate(
        [64 * (4 * t + r_ // 32) + 32 * g + (r_ % 32)
         for t in (0, 1) for g in (0, 1)]
    )

    def prep_qk(Wm):
        A = (WS * Wm)[cols, :].T  # [c, colpos]
        return np.ascontiguousarray(
            A.reshape(CT, P, DIM).transpose(1, 0, 2)
        ).astype(_F8)

    def prep_nat(Wm):
        A = (WS * Wm).T  # [c, o]
        return np.ascontiguousarray(
            A.reshape(CT, P, DIM).transpose(1, 0, 2)
        ).astype(_F8)

    shared = {
        "wq": prep_qk(Wq),
        "wk": prep_qk(Wk),
        "wv": prep_nat(Wv),
        "wo": prep_nat(Wo),
        "bq": np.ascontiguousarray(
            (WS * bq)[cols].reshape(CT, P).T
        ).astype(np.float32),
    }
    in_maps = []
    for b in range(B):
        m = dict(shared)
        m["x8"] = np.ascontiguousarray(
            xf[b].reshape(CT, P, N).transpose(1, 0, 2)
        ).astype(_F8)
        m["x32"] = xf[b] + rconst[:, None]
        in_maps.append(m)
    return in_maps


def kernel(x, Wq, bq, Wk, bk, Wv, bv, Wo, bo, _trace=False):
    from concourse.bass_utils import run_bass_kernel_spmd

    x = np.asarray(x)
    B, C, H, W = x.shape
    in_maps = _prep_maps(x, Wq, bq, Wk, bk, Wv, bv, Wo, bo)
    nc = build_nc()
    res = run_bass_kernel_spmd(nc, in_maps, core_ids=list(range(B)), trace=_trace)
    out = np.stack([res.results[b]["out"] for b in range(B)])
    out = out.reshape(B, C, H, W).astype(np.float32)
    if _trace:
        kernel.last_results = res
    return out
